# revision 15
# baseline (speedup 1.0000x reference)
"""DUQ RBF head kernel for Trainium2 (8 NeuronCores, batch-parallel).

Computes out[b,c,h,w] = exp(gamma * mean_e (einsum('bfhw,ecf', x, W) - m/N)^2)
for features [8,512,128,128], weights [16,64,512], m [16,64], N [64].

Strategy: data-parallel over batch (1 image per core). Per core, one big
matmul [ec=1024, f=512] @ [f=512, pix=16384] in float32r (full-rate fp32,
self-loading weights so the weight load hides behind the previous matmul;
measured faster than fp16/bf16 whose separate LDWEIGHTS serialize). Pixels
are processed in groups of 1024 (two 512-col PSUM banks per ec-chunk) so
the Square epilogue runs as wide ACT instructions with the centroid folded
into the per-partition bias; DVE accumulates over ec-chunks and does the
e-fold; ACT applies the final Exp. First/last groups are 512 wide to cut
startup latency and the drain tail; the first group's feature chunks load
k-at-a-time on a second DMA queue so the first matmul starts ASAP.
"""

import numpy as np

import concourse.bacc as bacc_mod
import concourse.mybir as mybir
import concourse.tile as tile
from concourse.bass_utils import run_bass_kernel_spmd

dt = mybir.dt
Act = mybir.ActivationFunctionType

B, F, H, W = 8, 512, 128, 128
E, C = 16, 64
PIX = H * W           # 16384 pixels per image
MCH = (E * C) // 128  # 8 ec-chunks of 128 partitions
KCH = F // 128        # 4 contraction chunks
LENGTH_SCALE = 0.1
GAMMA = -1.0 / (2.0 * LENGTH_SCALE**2)   # -50.0
EXP_SCALE = GAMMA / E                    # -3.125

# pixel groups: small first group (fast PE start), wide middle groups
# (amortized ACT/DVE instruction overhead), two small last groups (short
# drain tail)
GROUPS = [512] + [1024] * 15 + [256, 256]
assert sum(GROUPS) == PIX
GW = 1024  # allocation width; narrower groups use [:, :gw] slices
NWARM = 6  # junk matmuls to ramp the PE (HAM K-state) during DMA warmup


def _build():
    nc = bacc_mod.Bacc(None)
    feat_d = nc.declare_dram_parameter("feat", [F, PIX], dt.float32r, isOutput=False)
    wt_d = nc.declare_dram_parameter("wt", [F, E * C], dt.float32r, isOutput=False)
    negc_d = nc.declare_dram_parameter("negc", [128, MCH], dt.float32, isOutput=False)
    out_d = nc.declare_dram_parameter("out", [C, PIX], dt.float32, isOutput=True)

    feat_k = feat_d.rearrange("(k p) x -> p k x", k=KCH)
    wt_k = wt_d.rearrange("(k p) m -> p k m", k=KCH)

    with tile.TileContext(nc) as tc:
        with (
            tc.tile_pool(name="singles", bufs=1) as singles,
            tc.tile_pool(name="xin", bufs=4) as xin,
            tc.tile_pool(name="sqp", bufs=2) as sqp,
            tc.tile_pool(name="accp", bufs=2) as accp,
            tc.tile_pool(name="outp", bufs=2) as outp,
            tc.tile_pool(name="ps", bufs=4, space="PSUM") as ps,
        ):
            # negc rides the (otherwise idle) gpsimd queue
            negc_sb = singles.tile([128, MCH], dt.float32, tag="negc")
            nc.gpsimd.dma_start(out=negc_sb, in_=negc_d[:, :])

            ws = []
            for m in range(MCH):
                wsm = singles.tile([128, KCH, 128], dt.float32r, tag=f"ws{m}")
                ws.append(wsm)

            # PE pre-warm: junk matmuls with no DMA deps ramp the tensor
            # engine's clock-gate state while the first inputs stream in.
            junk = singles.tile([128, 512], dt.float32r, tag="junk")
            # memset rejects float32r; set the fp32 bit pattern of 1.0
            nc.vector.memset(junk[:, :].bitcast(dt.uint32), 0x3F800000)
            jps = ps.tile([128, GW], dt.float32, tag="mm")
            for _ in range(NWARM):
                nc.tensor.matmul(
                    out=jps[:, 0:512], lhsT=junk[:, 0:128], rhs=junk,
                    start=True, stop=True,
                )

            # first weight tile on the sync queue, group-0 x chunks and the
            # group-1 halves on the scalar queue: the DMA configs of the two
            # queues run in parallel so the first matmuls are not serialized
            # behind a single queue.
            g0w = GROUPS[0]
            xg0 = xin.tile([128, KCH, GW], dt.float32r, tag="x")
            xg1 = xin.tile([128, KCH, GW], dt.float32r, tag="x")
            nc.sync.dma_start(out=ws[0], in_=wt_k[:, :, 0:128])
            # group-0 x chunks split across the scalar and gpsimd queues so
            # the four transfers land at twice the single-queue cadence
            nc.scalar.dma_start(out=xg0[:, 0, 0:g0w], in_=feat_k[:, 0:1, 0:g0w])
            nc.gpsimd.dma_start(out=xg0[:, 2, 0:g0w], in_=feat_k[:, 2:3, 0:g0w])
            nc.scalar.dma_start(out=xg0[:, 1, 0:g0w], in_=feat_k[:, 1:2, 0:g0w])
            nc.gpsimd.dma_start(out=xg0[:, 3, 0:g0w], in_=feat_k[:, 3:4, 0:g0w])
            for m in range(1, MCH):
                nc.sync.dma_start(out=ws[m], in_=wt_k[:, :, m * 128 : (m + 1) * 128])
            px1 = slice(GROUPS[0], GROUPS[0] + GROUPS[1])
            nc.scalar.dma_start(out=xg1[:, 0:2, :], in_=feat_k[:, 0:2, px1])
            nc.scalar.dma_start(out=xg1[:, 2:4, :], in_=feat_k[:, 2:4, px1])

            px0 = 0
            for g, gw in enumerate(GROUPS):
                px = slice(px0, px0 + gw)
                if g == 0:
                    xg = xg0
                elif g == 1:
                    xg = xg1
                else:
                    xg = xin.tile([128, KCH, GW], dt.float32r, tag="x")
                    nc.sync.dma_start(out=xg[:, 0:2, 0:gw], in_=feat_k[:, 0:2, px])
                    nc.sync.dma_start(out=xg[:, 2:4, 0:gw], in_=feat_k[:, 2:4, px])

                segs = [
                    slice(t * 512, min((t + 1) * 512, gw))
                    for t in range((gw + 511) // 512)
                ]
                acc = accp.tile([128, GW], dt.float32, tag="acc")
                for m in range(MCH):
                    pst = ps.tile([128, GW], dt.float32, tag="mm")
                    for k in range(KCH):
                        for cs in segs:
                            nc.tensor.matmul(
                                out=pst[:, cs], lhsT=ws[m][:, k, :],
                                rhs=xg[:, k, cs],
                                start=(k == 0), stop=(k == KCH - 1),
                            )
                    if m == 0:
                        nc.scalar.activation(
                            out=acc[:, 0:gw], in_=pst[:, 0:gw], func=Act.Square,
                            bias=negc_sb[:, 0:1], scale=1.0,
                        )
                    else:
                        sq = sqp.tile([128, GW], dt.float32, tag="sq")
                        nc.scalar.activation(
                            out=sq[:, 0:gw], in_=pst[:, 0:gw], func=Act.Square,
                            bias=negc_sb[:, m : m + 1], scale=1.0,
                        )
                        nc.vector.tensor_add(
                            out=acc[:, 0:gw], in0=acc[:, 0:gw], in1=sq[:, 0:gw]
                        )

                tmp = outp.tile([64, GW], dt.float32, tag="tmp")
                nc.vector.tensor_copy(out=tmp[:, 0:gw], in_=acc[64:128, 0:gw])
                hc = outp.tile([64, GW], dt.float32, tag="hc")
                nc.vector.tensor_add(
                    out=hc[:, 0:gw], in0=acc[0:64, 0:gw], in1=tmp[:, 0:gw]
                )
                eo = outp.tile([64, GW], dt.float32, tag="eo")
                nc.scalar.activation(
                    out=eo[:, 0:gw], in_=hc[:, 0:gw], func=Act.Exp,
                    bias=0.0, scale=EXP_SCALE,
                )
                nc.scalar.dma_start(out=out_d[:, px], in_=eo[:, 0:gw])
                px0 += gw

    nc.finalize()
    return nc


_NC_CACHE = {}


def _get_nc():
    if "nc" not in _NC_CACHE:
        _NC_CACHE["nc"] = _build()
    return _NC_CACHE["nc"]


def _prep_inputs(features, weights, m, N):
    # wt[f, e*64+c] = weights[e, c, f]
    wt = np.ascontiguousarray(
        weights.astype(np.float32).transpose(2, 0, 1).reshape(F, E * C)
    )
    cent = (m.astype(np.float32) / N.astype(np.float32)[None, :]).reshape(-1)  # [ec]
    negc = np.ascontiguousarray(-cent.reshape(MCH, 128).T)  # [128, MCH]
    feats = np.ascontiguousarray(features.astype(np.float32).reshape(B, F, PIX))
    return [{"feat": feats[i], "wt": wt, "negc": negc} for i in range(B)]


def run_spmd(features, weights, m, N, trace=False):
    in_maps = _prep_inputs(features, weights, m, N)
    res = run_bass_kernel_spmd(_get_nc(), in_maps, list(range(B)), trace=trace)
    out = np.stack([res.results[i]["out"] for i in range(B)])  # [B, C, PIX]
    return out.reshape(B, C, H, W).astype(np.float32), res


def kernel(features, weights, m, N):
    out, _ = run_spmd(features, weights, m, N, trace=False)
    return out
